# revision 5
# baseline (speedup 1.0000x reference)
"""Trainium2 Bass kernel for nn_CRSDBlock — v1: time-parallel with burn-in.

Problem: B=32, T=1024, D=R=1024, 2 layers.
  per layer, per step t:
    r_new = (1-a)*r + a*tanh(x_t @ Wxr + h @ Whr)
    h_new = tanh(x_t @ Wxh + h @ Whh + r_new @ Wrh)

Key facts driving the design:
  1. The recurrence is PE weight-load bound: each step streams Whr+Whh+Wrh
     (192 chunk-pairs) through the PE; cost/step is ~independent of batch
     size up to b~32. So data-parallelism over batch wastes 7 of 8 cores.
  2. The recurrence is strongly contracting (echo-state property): a zero
     state restarted mid-sequence converges to the true trajectory
     (rel err ~5e-3 after 64 steps, ~9e-5 after 128).

Strategy (8 NeuronCores, NO collectives):
  - Time-parallel: core j owns output steps [s_j, s_j+CH), CH = T/8 = 128,
    with ALL 32 batch rows. It computes layer 1 over [s_j-E1-B2, s_j+CH)
    from a zero state (burn-in) and layer 2 over [s_j-B2, s_j+CH) using its
    own layer-1 output. Windows below t=0 are zero-padded by the host —
    zero inputs keep the state exactly zero, so core 0 is exact and the
    program is uniform across cores (pure SPMD, no role branching).
  - Host does all transposes: x is passed pre-transposed/bf16 as
    [128(part) , L1(t), 8(ki), 32(b)]; weights pre-converted to bf16
    (W_rh pre-scaled by ALPHA); output returned in device layout and
    un-transposed on the host. No on-device transpose phases.
  - Per layer: phase B precomputes XR/XH for the window with big N=512
    matmuls (fp16 DRAM scratch); phase C runs the recurrence with all
    operands feature-major (feat on partitions, batch on free axis),
    tracking s = r @ (ALPHA*Wrh) so the leak update is one add.
  - TensorE in bf16, fp32 PSUM accumulation.
"""

import numpy as np
import ml_dtypes

import concourse.bass as bass
import concourse.bacc as bacc_mod
import concourse.mybir as mybir
from concourse.tile import TileContext
from concourse.bass import ds
from concourse.bass_utils import run_bass_kernel_spmd

FP32 = mybir.dt.float32
FP16 = mybir.dt.float16
BF16 = mybir.dt.bfloat16
AF = mybir.ActivationFunctionType
ALU = mybir.AluOpType

P = 128
B32 = 32          # full batch on every core
D = 1024
NCH = D // P      # 8 feature chunks
ALPHA = 0.1
N_CORES = 8

TRACE = False
LAST_EXEC_NS = None


def build_nc(CH=64, E1=32, B2=64, SPB=16, NCOL=64):
    """CH: output steps per time-chunk; E1: extra layer-1 burn-in; B2:
    layer-2 burn-in; SPB: steps per For_i body (2 half-bodies of SUB
    each); NCOL: recurrence columns = 32 batch x (NCOL//32) independent
    time-chunks run in lockstep, sharing every weight load."""
    SUB = SPB // 2
    L1 = CH + E1 + B2
    L2 = CH + B2
    assert L1 % SPB == 0 and L2 % SPB == 0 and E1 % SUB == 0
    NH1 = L1 // SUB           # half-bodies in layer-1 window
    NH2 = L2 // SUB
    H_OFF = E1 // SUB         # first h1T half-body used by layer 2
    TPH = SUB * NCOL          # tokens per half-body (phase-B tile)
    assert TPH == 512

    nc = bacc_mod.Bacc(None)

    # All DRAM scratch/IO uses [P, half, ki, t_in_half, b] so every DMA in
    # the kernel is fully contiguous per partition (no 64B-fragment DMAs).
    xT = nc.declare_dram_parameter("xT", [P, NH1, NCH, SUB, NCOL], BF16,
                                   isOutput=False)
    Wxh = nc.declare_dram_parameter("W_xh", [2, D, D], BF16, isOutput=False)
    Whh = nc.declare_dram_parameter("W_hh", [2, D, D], BF16, isOutput=False)
    Wrh = nc.declare_dram_parameter("W_rh", [2, D, D], BF16, isOutput=False)
    Wxr = nc.declare_dram_parameter("W_xr", [2, D, D], BF16, isOutput=False)
    Whr = nc.declare_dram_parameter("W_hr", [2, D, D], BF16, isOutput=False)
    out = nc.declare_dram_parameter("out", [P, NH2, NCH, SUB, NCOL], BF16,
                                    isOutput=True)

    with TileContext(nc) as tc:
        with tc.tile_pool(name="dram", bufs=1, space="DRAM") as dram_pool:
            h1T = dram_pool.tile([P, NH1, NCH, SUB, NCOL], BF16)
            xr1 = dram_pool.tile([P, NH1, NCH, SUB, NCOL], FP16)
            xh1 = dram_pool.tile([P, NH1, NCH, SUB, NCOL], FP16)
            xr2 = dram_pool.tile([P, NH2, NCH, SUB, NCOL], FP16)
            xh2 = dram_pool.tile([P, NH2, NCH, SUB, NCOL], FP16)

            for layer in range(2):
                NH = NH1 if layer == 0 else NH2
                xr_d = xr1 if layer == 0 else xr2
                xh_d = xh1 if layer == 0 else xh2
                dstT = h1T if layer == 0 else out

                with tc.tile_pool(name=f"w{layer}", bufs=1) as wpool:
                    w = {}
                    # xr/xh weights first: phase B needs only these
                    for nm, srcw in (("xr", Wxr), ("xh", Wxh)):
                        wt = wpool.tile([P, NCH, D], BF16, tag=f"w_{nm}")
                        w[nm] = wt
                        for ki in range(NCH):
                            nc.sync.dma_start(
                                out=wt[:, ki, :],
                                in_=srcw[layer, ki * P:(ki + 1) * P, :])

                    # ---- Phase B: XR/XH over the whole window
                    with tc.tile_pool(name=f"pb{layer}", bufs=2) as pb_pool, \
                         tc.tile_pool(name=f"pbp{layer}", bufs=6,
                                      space="PSUM") as pbp_pool:
                        for h in range(NH):
                            xt = []
                            for ki in range(NCH):
                                tl = pb_pool.tile([P, SUB, NCOL], BF16,
                                                  tag="pb_rhs", bufs=2 * NCH)
                                if layer == 0:
                                    nc.sync.dma_start(
                                        out=tl, in_=xT[:, h, ki])
                                else:
                                    nc.sync.dma_start(
                                        out=tl, in_=h1T[:, H_OFF + h, ki])
                                xt.append(tl)
                            for nm, dst in (("xr", xr_d), ("xh", xh_d)):
                                for mo in range(NCH):
                                    ps = pbp_pool.tile([P, TPH], FP32,
                                                       tag="pb_ps")
                                    for ki in range(NCH):
                                        nc.tensor.matmul(
                                            ps, w[nm][:, ki, mo * P:(mo + 1) * P],
                                            xt[ki], start=(ki == 0),
                                            stop=(ki == NCH - 1))
                                    so = pb_pool.tile([P, TPH], FP16,
                                                      tag="pb_out", bufs=6)
                                    nc.vector.tensor_copy(so, ps)
                                    nc.sync.dma_start(
                                        out=dst[:, h, mo], in_=so)

                    # recurrence weights after phase B is emitted
                    for nm, srcw in (("hr", Whr), ("hh", Whh), ("rh", Wrh)):
                        wt = wpool.tile([P, NCH, D], BF16, tag=f"w_{nm}")
                        w[nm] = wt
                        for ki in range(NCH):
                            nc.sync.dma_start(
                                out=wt[:, ki, :],
                                in_=srcw[layer, ki * P:(ki + 1) * P, :])

                    # ---- Phase C: the sequential recurrence
                    with tc.tile_pool(name=f"st{layer}", bufs=1) as st_pool, \
                         tc.tile_pool(name=f"cb{layer}", bufs=1) as cb_pool, \
                         tc.tile_pool(name=f"cp{layer}", bufs=2,
                                      space="PSUM") as cp_pool, \
                         tc.tile_pool(name=f"cq{layer}", bufs=2) as cq_pool:
                        # hbuf holds SPB steps of tanh(h) in bf16: the block
                        # store source AND the recurrent matmul rhs history.
                        hbuf = st_pool.tile([P, 2, NCH, SUB, NCOL], BF16,
                                            tag="hbuf")
                        uT = st_pool.tile([P, NCH, NCOL], BF16, tag="uT")
                        s_sb = st_pool.tile([P, NCH, NCOL], FP32, tag="s")
                        nc.vector.memset(hbuf, 0.0)
                        nc.vector.memset(s_sb, 0.0)

                        NB = NH // 2
                        with tc.For_i(0, 2 * NB, 2,
                                      hint_engines=(mybir.EngineType.PE,)) as bv:
                            xrb0 = cb_pool.tile([P, NCH, SUB, NCOL], FP16,
                                                tag="xrb0")
                            xrb1 = cb_pool.tile([P, NCH, SUB, NCOL], FP16,
                                                tag="xrb1")
                            xhb0 = cb_pool.tile([P, NCH, SUB, NCOL], FP16,
                                                tag="xhb0")
                            xhb1 = cb_pool.tile([P, NCH, SUB, NCOL], FP16,
                                                tag="xhb1")
                            nc.sync.dma_start(out=xrb0, in_=xr_d[:, ds(bv, 1)])
                            nc.sync.dma_start(out=xhb0, in_=xh_d[:, ds(bv, 1)])
                            nc.sync.dma_start(out=xrb1, in_=xr_d[:, ds(bv + 1, 1)])
                            nc.sync.dma_start(out=xhb1, in_=xh_d[:, ds(bv + 1, 1)])

                            for j in range(SPB):
                                half, jj = j // SUB, j % SUB
                                ph, pj = ((j - 1) // SUB, (j - 1) % SUB) \
                                    if j > 0 else (1, SUB - 1)
                                h_in = hbuf[:, ph, :, pj]   # [P, NCH, B32]
                                xrb = xrb0 if half == 0 else xrb1
                                xhb = xhb0 if half == 0 else xhb1
                                xr_j = xrb[:, :, jj]
                                xh_j = xhb[:, :, jj]

                                psu = cp_pool.tile([P, NCH, NCOL], FP32,
                                                   tag="psu", bufs=1)
                                for mo in range(NCH):
                                    for ki in range(NCH):
                                        nc.tensor.matmul(
                                            psu[:, mo, :],
                                            w["hr"][:, ki, mo * P:(mo + 1) * P],
                                            h_in[:, ki, :], start=(ki == 0),
                                            stop=(ki == NCH - 1))
                                psh = cp_pool.tile([P, NCH, NCOL], FP32,
                                                   tag="psh", bufs=1)
                                for mo in range(NCH):
                                    for ki in range(NCH):
                                        nc.tensor.matmul(
                                            psh[:, mo, :],
                                            w["hh"][:, ki, mo * P:(mo + 1) * P],
                                            h_in[:, ki, :], start=(ki == 0),
                                            stop=(ki == NCH - 1))
                                # u = tanh(psu + xr_j): the add is emitted
                                # FIRST on the DVE so the u-chain isn't
                                # delayed behind s09/q bookkeeping.
                                nc.vector.tensor_add(psu, psu, xr_j)
                                nc.scalar.activation(uT, psu, AF.Tanh)
                                # q = psh + xh_j + 0.9*s_prev; the leak
                                # scale is fused into the second add via
                                # scalar_tensor_tensor, eliminating the
                                # separate 0.9*s op.
                                q_sb = cq_pool.tile([P, NCH, NCOL], FP32,
                                                    tag="q")
                                nc.vector.tensor_add(q_sb, psh, xh_j)
                                nc.vector.scalar_tensor_tensor(
                                    q_sb, s_sb, 1.0 - ALPHA, q_sb,
                                    ALU.mult, ALU.add)
                                # pss = u @ (a*Wrh) in 4 mo-quarters with
                                # independent PSUM banks: earlier quarters'
                                # add/tanh overlap later quarters' matmuls.
                                QCH = NCH // 4
                                for qu in range(4):
                                    pt = cp_pool.tile([P, QCH, NCOL], FP32,
                                                      tag=f"pss{qu}", bufs=1)
                                    for m in range(QCH):
                                        mo = qu * QCH + m
                                        for ki in range(NCH):
                                            nc.tensor.matmul(
                                                pt[:, m, :],
                                                w["rh"][:, ki, mo * P:(mo + 1) * P],
                                                uT[:, ki, :], start=(ki == 0),
                                                stop=(ki == NCH - 1))
                                    sl = slice(qu * QCH, (qu + 1) * QCH)
                                    hq = cq_pool.tile([P, QCH, NCOL], FP32,
                                                      tag=f"hq{qu}", bufs=1)
                                    nc.vector.tensor_add(hq, pt, q_sb[:, sl])
                                    nc.scalar.activation(
                                        hbuf[:, half, sl, jj], hq, AF.Tanh)
                                    nc.vector.scalar_tensor_tensor(
                                        s_sb[:, sl], s_sb[:, sl],
                                        1.0 - ALPHA, pt, ALU.mult, ALU.add)
                                # store each half as soon as it completes
                                if jj == SUB - 1:
                                    hv = bv if half == 0 else bv + 1
                                    nc.sync.dma_start(out=dstT[:, ds(hv, 1)],
                                                      in_=hbuf[:, half])

    nc.finalize()
    return nc


def _to_bf16(a):
    return np.ascontiguousarray(a.astype(ml_dtypes.bfloat16))


def kernel(x_seq, W_xh, W_hh, W_rh, W_xr, W_hr):
    global LAST_EXEC_NS
    B, T, Dd = x_seq.shape
    CPC = 2                     # time-chunks per core (lockstep columns)
    NCHK = N_CORES * CPC        # 16 chunks total
    CH = T // NCHK              # 64 output steps per chunk
    E1, B2 = 16, 64
    L1 = CH + E1 + B2
    SUB = 8
    NH1 = L1 // SUB
    nc = build_nc(CH=CH, E1=E1, B2=B2, SPB=2 * SUB, NCOL=CPC * B)

    wb = {
        "W_xh": _to_bf16(W_xh),
        "W_hh": _to_bf16(W_hh),
        "W_rh": _to_bf16(W_rh * ALPHA),
        "W_xr": _to_bf16(W_xr),
        "W_hr": _to_bf16(W_hr),
    }
    # per-chunk windows: [s_g - E1 - B2, s_g + CH), zero-padded below t=0
    pad = E1 + B2
    x_pad = np.concatenate(
        [np.zeros((B, pad, Dd), np.float32), x_seq], axis=1)
    in_maps = []
    for j in range(N_CORES):
        cols = []
        for c in range(CPC):
            g = j * CPC + c
            w = x_pad[:, g * CH:g * CH + L1]                   # [B, L1, D]
            # -> [P, half, ki, t_in_half, b]
            cols.append(w.reshape(B, NH1, SUB, NCH, P)
                        .transpose(4, 1, 3, 2, 0))
        arr = np.concatenate(cols, axis=4)      # [P, NH1, ki, SUB, CPC*B]
        m = {"xT": _to_bf16(arr)}
        m.update(wb)
        in_maps.append(m)

    res = run_bass_kernel_spmd(nc, in_maps, core_ids=list(range(N_CORES)),
                               trace=TRACE)
    LAST_EXEC_NS = res.exec_time_ns

    H_OFF = B2 // SUB
    out_full = np.empty((B, T, Dd), np.float32)
    for j in range(N_CORES):
        o = np.asarray(res.results[j]["out"]).astype(np.float32)
        # [P, NH2, NCH, SUB, CPC*B] -> keep halves >= H_OFF
        o = o[:, H_OFF:]
        for c in range(CPC):
            g = j * CPC + c
            oc = o[:, :, :, :, c * B:(c + 1) * B]
            oc = oc.transpose(4, 1, 3, 2, 0).reshape(B, CH, Dd)
            out_full[:, g * CH:(g + 1) * CH] = oc
    return out_full
